# revision 1
# baseline (speedup 1.0000x reference)
"""Fully-fused fp16 MoE expert FFN (E=8, C=2048, D=1024, F=4096), 8 TRN2 cores.

One expert per core. w1 AND w2 are fully SBUF-resident in fp16
(64KB/partition each). Per 512-token chunk: mm1 (32 psum groups, gelu+b1
fused in ACT eviction) -> hT chunk in SBUF -> mm2 (8 psum groups, K=4096
accumulation) -> out. No DRAM intermediates at all; total HBM traffic
28MB/core. PE sees one continuous stream of 2048 N=512 matmuls at
1 cycle/row (fp16).
"""

import numpy as np

import concourse.bass as bass
import concourse.mybir as mybir
import concourse.tile as tile
from concourse import bacc
from concourse.bass_utils import run_bass_kernel_spmd

E, C, D, F = 8, 2048, 1024, 4096
P = 128
KD = D // P  # 8
MF = F // P  # 32
CN = C // 512  # 4 chunks of 512 tokens
CJ = 4  # 128-token subblocks per chunk
DN = D // 512  # 2

F32 = mybir.dt.float32
F16 = mybir.dt.float16
GELU = mybir.ActivationFunctionType.Gelu_apprx_tanh

_CACHE = {}


def _build():
    nc = bacc.Bacc("TRN2", target_bir_lowering=False, debug=False, num_devices=E)

    xT_d = nc.dram_tensor("xT", [KD, P, C], F16, kind="ExternalInput").ap()
    w1_d = nc.dram_tensor("w1r", [KD, P, F], F16, kind="ExternalInput").ap()
    b1_d = nc.dram_tensor("b1t", [P, MF], F32, kind="ExternalInput").ap()
    w2_d = nc.dram_tensor("w2r", [MF, P, D], F16, kind="ExternalInput").ap()
    out_d = nc.dram_tensor("out", [C, D], F32, kind="ExternalOutput").ap()

    with tile.TileContext(nc) as tc:
        with (
            tc.tile_pool(name="w1f", bufs=1) as w1_pool,
            tc.tile_pool(name="w2f", bufs=1) as w2_pool,
            tc.tile_pool(name="b1", bufs=1) as b1_pool,
            tc.tile_pool(name="xt", bufs=2) as xt_pool,
            tc.tile_pool(name="ht", bufs=1) as ht_pool,
            tc.tile_pool(name="ev", bufs=4) as ev_pool,
            tc.tile_pool(name="ps1", bufs=4, space="PSUM") as ps1_pool,
            tc.tile_pool(name="ps2", bufs=4, space="PSUM") as ps2_pool,
        ):
            b1t = b1_pool.tile([P, MF], F32)
            nc.sync.dma_start(b1t[:], b1_d[:])

            def load_xt(cn):
                t = xt_pool.tile([P, KD, 512], F16, tag="xt")
                for k in range(KD):
                    nc.sync.dma_start(
                        t[:, k, :], xT_d[k, :, cn * 512 : (cn + 1) * 512]
                    )
                return t

            # chunk-0 activations first so the PE can start immediately;
            # then w1 in F-column-major pieces (psum group j needs column
            # block j for all k), then w2 ordered by dn-half (mm2 group
            # (cj, dn) reads the dn half of every fk row).
            xt0 = load_xt(0)

            w1f = w1_pool.tile([P, KD, F], F16)
            for jj in range(F // 512):
                for k in range(KD):
                    nc.sync.dma_start(
                        w1f[:, k, bass.ds(jj * 512, 512)],
                        w1_d[k, :, jj * 512 : (jj + 1) * 512],
                    )
            w2f = w2_pool.tile([P, MF, D], F16)
            for dn in range(DN):
                for j in range(MF):
                    nc.sync.dma_start(
                        w2f[:, j, bass.ds(dn * 512, 512)],
                        w2_d[j, :, dn * 512 : (dn + 1) * 512],
                    )

            for cn in range(CN):
                xt = xt0 if cn == 0 else load_xt(cn)
                ht = ht_pool.tile([P, MF, 512], F16, tag="ht")
                for j in range(MF):
                    ps = ps1_pool.tile([P, 512], F32, tag="ps1")
                    for k in range(KD):
                        nc.tensor.matmul(
                            ps[:],
                            w1f[:, k, bass.ds(j * P, P)],
                            xt[:, k, :],
                            start=(k == 0),
                            stop=(k == KD - 1),
                        )
                    nc.scalar.activation(
                        ht[:, j, :], ps[:], GELU, bias=b1t[:, j : j + 1]
                    )
                for cj in range(CJ):
                    row = cn * 512 + cj * P
                    for dn in range(DN):
                        ps = ps2_pool.tile([P, 512], F32, tag="ps2")
                        for j in range(MF):
                            nc.tensor.matmul(
                                ps[:],
                                ht[:, j, bass.ds(cj * P, P)],
                                w2f[:, j, bass.ds(dn * 512, 512)],
                                start=(j == 0),
                                stop=(j == MF - 1),
                            )
                        ev = ev_pool.tile([P, 512], F32, tag="ev")
                        nc.vector.tensor_copy(ev[:], ps[:])
                        nc.sync.dma_start(
                            out_d[row : row + P, dn * 512 : (dn + 1) * 512],
                            ev[:],
                        )

    nc.compile()
    return nc


def _get_nc():
    if "nc" not in _CACHE:
        _CACHE["nc"] = _build()
    return _CACHE["nc"]


def _in_map(x_e, w1_e, b1_e, w2_e):
    xT = np.ascontiguousarray(x_e.T).astype(np.float16).reshape(KD, P, C)
    w1r = w1_e.astype(np.float16).reshape(KD, P, F)
    b1t = np.ascontiguousarray(b1_e.reshape(MF, P).T)
    w2r = w2_e.astype(np.float16).reshape(MF, P, D)
    return {"xT": xT, "w1r": w1r, "b1t": b1t, "w2r": w2r}


def kernel(inputs, w1, b1, w2, b2, _trace=False):
    nc = _get_nc()
    x = np.asarray(inputs, dtype=np.float32).reshape(E, C, D)
    in_maps = [
        _in_map(
            x[e],
            np.asarray(w1[e], dtype=np.float32),
            np.asarray(b1[e], dtype=np.float32),
            np.asarray(w2[e], dtype=np.float32),
        )
        for e in range(E)
    ]
    res = run_bass_kernel_spmd(nc, in_maps, list(range(E)), trace=_trace)
    out = np.stack([res.results[e]["out"] for e in range(E)])[None]
    out = out + np.asarray(b2, dtype=np.float32)[None]
    if _trace:
        _CACHE["last_results"] = res
    return out.astype(np.float32)

